# revision 67
# baseline (speedup 1.0000x reference)
"""Multi-head causal attention (B=4, T=2048, C=1024, 16 heads) on 8 trn2 cores.

Sharding: core c handles batch b = c//2 and head-group g = c%2 (8 heads).
Each core computes qkv projection, causal attention and its c_proj partial
product for its 512 attention channels; the host sums the two partials per
batch and adds b_proj.

On-device layout (all matmul inputs bf16, fp32 PSUM accumulation):
  - x is fed pre-transposed per batch: xT [C, T].
  - qT, kT computed as W.T @ xT -> [head_dim, T] per head, two heads packed
    per 128 partitions ("pair" tiles).
  - v computed naturally [T, head_dim*8] with a ones column PREPENDED per
    head so the attn@v matmul also produces the softmax denominators.
  - scores computed transposed (k tokens on partitions, q free); exp on
    ScalarE with the 1/sqrt(64) scale folded in; causal mask via packed bf16
    mask multiplies on the diagonal tiles only.
  - attn@v runs probs-STATIONARY: per (q-subchunk, head) the [128k,128q]
    probs slice is the stationary operand and [ones|v] (65 cols) the moving
    one, so a chunk-pair costs 8x65=520 PE rows instead of 2x512 (the cost
    of a matmul is its moving size only). Output lands q-partitioned
    [q, head, 65] with the denominator in column 0 of each group;
    normalization is then a per-partition reciprocal + 8 tensor_scalar
    multiplies on DVE (no gpsimd broadcast chain), and 4 PE transposes
    ([128,128] bf16, 128 rows each) restore the [ch, tok] layout c_proj
    needs. Transposes ride the mm psum ring and are emitted one PE-chain
    after the pair so the DVE normalize hides under real PE work.
  - qkv / attention / c_proj are interleaved per 512-token block so PE, ACT
    and DVE load stays flat; DMAs are few and large (fixed ~625ns HWDGE
    descriptor-generation cost per dma_start) and alternate between the two
    HWDGE rings (SP / Activation).
"""

import sys

if "/opt/trn_rl_repo" not in sys.path:
    sys.path.insert(0, "/opt/trn_rl_repo")

from collections import deque
from contextlib import ExitStack

import numpy as np
import ml_dtypes

B, T, C = 4, 2048, 1024
H, D = 16, 64
HPG = 8          # heads per group (per core)
GC = HPG * D     # attention channels per core (512)
N_CORES = 8
KC = C // 128    # 8 contraction chunks over C
NQ = T // 512    # 4 q/token blocks
NT = T // 128    # 16 k chunks / token tiles

# diagonal-chunk q-window widths per offset j; only the first 128 columns of
# each window are causally mixed (the rest are fully visible)
MASK_W = [512, 384, 256, 128]

BF16 = ml_dtypes.bfloat16

_cached_nc = None
_runner = None
LAST_RESULTS = None


def _build_nc():
    import concourse.bacc as bacc
    import concourse.tile as tile
    from concourse import mybir
    from concourse.bass import broadcast_tensor_aps

    f32 = mybir.dt.float32
    bf16 = mybir.dt.bfloat16
    EXP = mybir.ActivationFunctionType.Exp

    nc = bacc.Bacc("TRN2", target_bir_lowering=False)

    xt = nc.dram_tensor("xt", [C, T], bf16, kind="ExternalInput")
    # [wk | wq | wv] column blocks of 512
    wqkv = nc.dram_tensor("wqkv", [C, 3 * GC], bf16, kind="ExternalInput")
    wp = nc.dram_tensor("wp", [GC, C], bf16, kind="ExternalInput")
    # cols 0-3: b_k pair columns, 4-7: b_q pair columns, 8-11: b_v pair
    # columns (b_v is added at the oT eviction: bias*denom/denom is exact)
    bqk = nc.dram_tensor("bqk", [128, 12], f32, kind="ExternalInput")
    # identity for PE transposes
    ident = nc.dram_tensor("ident", [128, 128], bf16, kind="ExternalInput")
    # bf16 partials: the host/pairsum upcasts to f32 before summing the two
    # half-head partials, so the extra rounding stays ~0.3% of absmax
    out = nc.dram_tensor("out", [T, C], bf16, kind="ExternalOutput")

    with tile.TileContext(nc) as tc, ExitStack() as ctx:
        pp = ctx.enter_context(tc.tile_pool(name="persist", bufs=1))
        xt_sb = pp.tile([128, KC, T], bf16, name="xt_sb")
        wqkv_sb = pp.tile([128, KC, 3 * GC], bf16, name="wqkv_sb")
        wp_sb = pp.tile([128, 4, C], bf16, name="wp_sb")
        bqk_sb = pp.tile([128, 12], f32, name="bqk_sb")
        ident_sb = pp.tile([128, 128], bf16, name="ident_sb")
        mask_sb = pp.tile([128, 128], bf16, name="mask_sb")
        qT_sb = pp.tile([128, 4, T], bf16, name="qT_sb")
        kT_sb = pp.tile([128, 4, T], bf16, name="kT_sb")
        # per k-chunk / head: [1.0 | v d0..d63 | pad] (66 for 4B alignment)
        v_sb = pp.tile([128, NT, HPG, 66], bf16, name="v_sb")
        oT_sb = pp.tile([128, 4, T], bf16, name="oT_sb")

        # few, large DMAs (HWDGE pays ~625ns per dma_start), alternating
        # between the two HWDGE rings; ordered first-needed-first.
        _dma_engs = [nc.sync, nc.scalar]
        _dma_i = [0]

        def dma(dst, src):
            _dma_engs[_dma_i[0] % 2].dma_start(dst, src)
            _dma_i[0] += 1

        # few, LARGE DMAs: the ~625ns HWDGE (and ~1.1us SWDGE) fixed cost per
        # dma_start dominates a chunked ladder, so 4-chunk slices go out as
        # single transfers. wk rides the gpsimd SWDGE pipe so both DGE paths
        # fill in parallel during the DMA-paced kernel start; first-needed
        # first: wk + first xt quarter (kT chains), then wq, wv, rest of xt.
        for half in range(2):
            kc0 = half * 4
            nc.gpsimd.dma_start(
                wqkv_sb[:, kc0:kc0 + 4, 0:512],
                wqkv[kc0 * 128:(kc0 + 4) * 128, 0:512])
            dma(xt_sb[:, kc0:kc0 + 4, 0:512],
                xt[kc0 * 128:(kc0 + 4) * 128, 0:512])
            if half == 0:
                dma(bqk_sb[:, :], bqk[:, :])
        for half in range(2):
            kc0 = half * 4
            dma(wqkv_sb[:, kc0:kc0 + 4, 1024:1536],
                wqkv[kc0 * 128:(kc0 + 4) * 128, 1024:1536])
        for half in range(2):
            kc0 = half * 4
            dma(wqkv_sb[:, kc0:kc0 + 4, 512:1024],
                wqkv[kc0 * 128:(kc0 + 4) * 128, 512:1024])
        for half in range(2):
            kc0 = half * 4
            dma(xt_sb[:, kc0:kc0 + 4, 512:2048],
                xt[kc0 * 128:(kc0 + 4) * 128, 512:2048])
        dma(ident_sb[:, :], ident[:, :])
        # NOTE: full-width dst slices must stay one 128-row chunk per DMA —
        # a combined [128, 2, 1024] dst merges contiguous and repairs rows
        for cc in range(4):
            dma(wp_sb[:, cc, :], wp[cc * 128:(cc + 1) * 128, :])
        warm_w = pp.tile([1, 128], bf16, name="warm_w")
        nc.vector.memset(warm_w[:, :], 1.0)
        nc.vector.memset(v_sb[:, :, :, 0:1], 1.0)

        # [128,128] causal mask (1.0 at x <= y) built on gpsimd (idle at
        # kernel start) instead of shipping it over the link
        nc.gpsimd.memset(mask_sb[:, :], 0.0)
        nc.gpsimd.affine_select(
            out=mask_sb[:, :],
            in_=mask_sb[:, :],
            compare_op=mybir.AluOpType.is_gt,
            fill=1.0,
            base=0,
            # keep where x - y > 0 is false -> fill 1.0 at x <= y
            pattern=[[-1, 128]],
            channel_multiplier=1,
        )

        with (
            tc.tile_pool(name="mm_ps", bufs=2, space="PSUM") as mmp,
            tc.tile_pool(name="sc_ps", bufs=2, space="PSUM") as scp,
            tc.tile_pool(name="o_ps", bufs=1, space="PSUM") as op,
            tc.tile_pool(name="probs", bufs=31) as prp,
            tc.tile_pool(name="norm", bufs=4) as nop,
            tc.tile_pool(name="ostage", bufs=6) as osp,
        ):
            # ---- filler queue: qkv/c_proj/transpose PE work is emitted one
            # matmul at a time INSIDE the ACT-paced attention pairs, so the
            # PE never starves while ScalarE chews through the exps. FIFO
            # with head-only stepping keeps at most one mm-ring generator
            # mid-flight (ring stays race-free).
            fillq = deque()
            done_tags = set()

            def push(tag, gen):
                fillq.append((tag, gen))

            def push_next(tag, gen):
                # behind the (possibly mid-flight) head, ahead of the rest
                if fillq:
                    fillq.insert(1, (tag, gen))
                else:
                    fillq.append((tag, gen))

            def step_fill(n):
                while n > 0 and fillq:
                    tag, g = fillq[0]
                    try:
                        next(g)
                        n -= 1
                    except StopIteration:
                        done_tags.add(tag)
                        fillq.popleft()

            def finish_through(tag):
                while tag not in done_tags:
                    assert fillq, f"filler {tag} never queued"
                    t, g = fillq[0]
                    for _ in g:
                        pass
                    done_tags.add(t)
                    fillq.popleft()

            def flush_all():
                while fillq:
                    t, g = fillq[0]
                    for _ in g:
                        pass
                    done_tags.add(t)
                    fillq.popleft()

            def qk_chain_gen(which, dst, j, nb):
                # psum[pair dims, tokens] = W_chunk.T @ xT_chunk
                ps = mmp.tile([128, 512], f32, name="ps_qk", tag="m")
                w0 = which * 512 + j * 128
                for kc in range(KC):
                    nc.tensor.matmul(
                        ps[:, :],
                        wqkv_sb[:, kc, w0:w0 + 128],
                        xt_sb[:, kc, nb * 512:(nb + 1) * 512],
                        start=(kc == 0),
                        stop=(kc == KC - 1),
                    )
                    if kc == KC - 1:
                        nc.vector.tensor_scalar_add(
                            dst[:, j, nb * 512:(nb + 1) * 512], ps[:, :],
                            bqk_sb[:, which * 4 + j:which * 4 + j + 1],
                        )
                    yield

            def v_chain_gen(tb):
                # psum[tokens, 8*64] = xT_chunk.T @ wv_chunk (+ bias row)
                ps = mmp.tile([128, 512], f32, name="ps_v", tag="m")
                for kc in range(KC):
                    nc.tensor.matmul(
                        ps[:, :],
                        xt_sb[:, kc, tb * 128:(tb + 1) * 128],
                        wqkv_sb[:, kc, 1024:1536],
                        start=(kc == 0),
                        stop=(kc == KC - 1),
                    )
                    if kc == KC - 1:
                        nc.vector.tensor_copy(
                            v_sb[:, tb, :, 1:65],
                            ps[:, :].rearrange("p (h d) -> p h d", h=HPG),
                        )
                    yield

            def cproj_gen(tb, split_dma=False, fine_tail=False):
                ost = osp.tile([128, 1024], bf16, name="ost", tag="ost")
                for nh in range(2):
                    c_ps = mmp.tile([128, 512], f32, name="c_acc", tag="m")
                    for cc in range(4):
                        nc.tensor.matmul(
                            c_ps[:, :],
                            oT_sb[:, cc, tb * 128:(tb + 1) * 128],
                            wp_sb[:, cc, nh * 512:(nh + 1) * 512],
                            start=(cc == 0),
                            stop=(cc == 3),
                        )
                        if cc == 3:
                            if fine_tail and nh == 1:
                                # the kernel-ending latency chain is
                                # evict+HWDGE+transfer+sem of the LAST piece:
                                # quarter-size it so the chain is shorter
                                for qtr in range(2):
                                    sl = slice(512 + qtr * 256,
                                               768 + qtr * 256)
                                    nc.vector.tensor_copy(
                                        ost[:, sl], c_ps[:, qtr * 256:
                                                         qtr * 256 + 256])
                                    dma(out[tb * 128:(tb + 1) * 128, sl],
                                        ost[:, sl])
                            else:
                                nc.vector.tensor_copy(
                                    ost[:, nh * 512:(nh + 1) * 512],
                                    c_ps[:, :])
                                if split_dma or fine_tail:
                                    dma(out[tb * 128:(tb + 1) * 128,
                                            nh * 512:(nh + 1) * 512],
                                        ost[:, nh * 512:(nh + 1) * 512])
                                elif nh == 1:
                                    dma(out[tb * 128:(tb + 1) * 128, :],
                                        ost[:, :])
                        yield

            def drain_gen(hp, q0, onrm):
                # [q, ch] -> [ch, q] PE transposes of the normalized window:
                # all four subchunks into ONE psum tile, one DVE eviction
                # (which also adds b_v: bias*denom/denom is exact)
                tT = mmp.tile([128, 4, 128], bf16, name="tT", tag="m")
                for sub in range(4):
                    h2 = 2 * (sub % 2)
                    nc.tensor.transpose(
                        tT[:, sub, :],
                        onrm[:, sub // 2, h2:h2 + 2, :], ident_sb[:, :])
                    if sub == 3:
                        nc.vector.tensor_scalar_add(
                            oT_sb[:, hp, q0:q0 + 512],
                            tT[:, :, :].rearrange("p s q -> p (s q)"),
                            bqk_sb[:, 8 + hp:9 + hp])
                    if sub % 2 == 1:
                        yield

            def attn_pair(qb, hp):
                q0 = qb * 512
                kT0 = kT_sb[0:64, hp, :]
                kT1 = kT_sb[64:128, hp, :]
                qT0 = qT_sb[0:64, hp, :]
                qT1 = qT_sb[64:128, hp, :]
                # [den|64 v-dims] groups of 65 f32; 4 groups per psum bank
                # (a matmul output must not cross a bank boundary)
                o = op.tile([128, 2, 512], f32, name="o_acc", tag="o")
                o4 = o[:, :, 0:260].rearrange("p b (g c) -> p b g c", c=65)
                n_full = 4 * qb

                def s_pair(s_ps, kc, qoff, n):
                    # both heads' K=64 matmuls back to back: rows 0-63
                    # and 64-127; h1's window bank-aligned at column 512
                    nc.tensor.matmul(
                        s_ps[:, 0:n], kT0[:, kc * 128:(kc + 1) * 128],
                        qT0[:, q0 + qoff:q0 + qoff + n],
                        start=True, stop=True,
                    )
                    nc.tensor.matmul(
                        s_ps[:, 512:512 + n], kT1[:, kc * 128:(kc + 1) * 128],
                        qT1[:, q0 + qoff:q0 + qoff + n],
                        start=True, stop=True,
                    )

                def o_mms(pr, kc, j):
                    # probs-stationary attn@v: [128,65] out per (sub, head).
                    # PSUM start/stop are per 2KB zero-region (bank): the
                    # bank's first write starts it (marking every byte
                    # pending-zero, so sibling groups' first writes zero
                    # correctly) and its last write stops it.
                    for sub in range(j, 4):
                        poff = (sub - j) * 128
                        for h in range(2):
                            g = 2 * sub + h
                            nc.tensor.matmul(
                                o4[:, g // 4, g % 4, :],
                                pr[:, 512 * h + poff:512 * h + poff + 128],
                                v_sb[:, kc, 2 * hp + h, 0:65],
                                start=(kc == 0 and g % 4 == 0),
                                stop=(h == 1 and sub % 2 == 1
                                      and kc == n_full + sub),
                            )

                pend = deque()

                def o_flush(depth):
                    while len(pend) > depth:
                        o_mms(*pend.popleft())

                for kc in range(n_full):
                    s = scp.tile([128, 1024], f32, name="s_t", tag="s")
                    s_pair(s, kc, 0, 512)
                    pr = prp.tile([128, 1024], bf16, name="pr", tag="pr")
                    nc.scalar.activation(pr[:, :], s[:, :], EXP, scale=0.125)
                    step_fill(2)
                    o_flush(2)
                    pend.append((pr, kc, 0))
                # [half][group%4][64] bf16, densely = the flat [q, 512ch]
                # normalized window (half = subs 0-1 vs 2-3)
                onrm = nop.tile([128, 2, 4, 64], bf16, name="onrm",
                                tag="onrm")

                # kT/v chains of this block feed the diagonal chunks: force
                # any unconsumed remainder out before the j-loop emits reads
                finish_through(("v", qb, 4 * qb + 3))
                for j in range(4):
                    kc = n_full + j
                    w = MASK_W[j]
                    qoff = 512 - w
                    s = scp.tile([128, 1024], f32, name="s_d", tag="s")
                    s_pair(s, kc, qoff, w)
                    pr = prp.tile([128, 1024], bf16, name="pr_d", tag="pr")
                    # one strided activation covers both heads' w-wide
                    # windows (h1 bank-aligned at column 512)
                    sv = s[:, :].rearrange("p (b c) -> p b c", b=2)
                    pv = pr[:, :].rearrange("p (b c) -> p b c", b=2)
                    nc.scalar.activation(
                        pv[:, :, 0:w], sv[:, :, 0:w], EXP, scale=0.125)
                    # only the first 128 columns of each head's window mix;
                    # the rest are fully visible (one strided multiply
                    # covers both heads' windows)
                    prm = pr[:, :].rearrange("p (b c) -> p b c", b=2)[:, :,
                                                                     0:128]
                    mask_b, _ = broadcast_tensor_aps(
                        mask_sb[:, :].rearrange("p (b c) -> p b c", b=1),
                        prm)
                    nc.vector.tensor_mul(prm, prm, mask_b)
                    step_fill(2 if j < 2 else 3)
                    o_flush(2)
                    pend.append((pr, kc, j))
                o_flush(0)

                # per-partition normalize on DVE: denominators sit in column
                # 0 of each [65] group; one broadcast multiply normalizes
                # the whole window
                rcp = nop.tile([128, 2, 4, 1], f32, name="rcp", tag="rcp")
                nc.vector.reciprocal(rcp[:, :, :, :], o4[:, :, :, 0:1])
                rcp_b, o_data = broadcast_tensor_aps(rcp[:, :, :, 0:1],
                                                     o4[:, :, :, 1:65])
                nc.vector.tensor_mul(onrm[:, :, :, :], o_data, rcp_b)
                push_next(("drain", qb, hp), drain_gen(hp, q0, onrm))

            def warm(n):
                # dummy matmuls during the DMA-paced kernel start keep the
                # PE p-state ramp warm (uses the still-idle scores slots)
                wp_ps = scp.tile([128, 1024], f32, name="warm_ps", tag="s")
                for _ in range(n):
                    nc.tensor.matmul(
                        wp_ps[:, 0:128], warm_w[0:1, :], warm_w[0:1, :],
                        start=True, stop=True,
                    )

            # Per token block nb: kT/v/qT chains and c_proj tiles of the
            # previous block stream through the filler queue, consumed at
            # ~2 matmuls per attention chunk.
            warm(40)
            for j in range(4):
                push(("kT", 0, j), qk_chain_gen(0, kT_sb, j, 0))
            for tb in range(4):
                push(("v", 0, tb), v_chain_gen(tb))
            push(("qT", 0, 0), qk_chain_gen(1, qT_sb, 0, 0))
            finish_through(("qT", 0, 0))
            for nb in range(NQ):
                for hpn in (1, 2, 3):
                    push(("qT", nb, hpn), qk_chain_gen(1, qT_sb, hpn, nb))
                # c_proj of block qb is held until phase qb+2: the last
                # block's pairs have the deepest ACT-paced stretches and need
                # the most PE filler
                for qb_c in range(NQ - 1):
                    if min(qb_c + 3, NQ - 1) == nb:
                        for i in range(4):
                            # cproj(NQ-2, 3) is held out of the queue: it
                            # covers the last pair's normalize drain below
                            if (qb_c, i) != (NQ - 2, 3):
                                push(("cproj", qb_c, i),
                                     cproj_gen(4 * qb_c + i))
                if nb + 1 < NQ:
                    push(("qT", nb + 1, 0), qk_chain_gen(1, qT_sb, 0, nb + 1))
                    for j in range(4):
                        push(("kT", nb + 1, j),
                             qk_chain_gen(0, kT_sb, j, nb + 1))
                    for tb in range(4 * (nb + 1), 4 * (nb + 1) + 4):
                        push(("v", nb + 1, tb), v_chain_gen(tb))
                for hp in range(4):
                    if hp > 0:
                        finish_through(("qT", nb, hp))
                    attn_pair(nb, hp)
                if nb + 1 < NQ:
                    finish_through(("qT", nb + 1, 0))
            for _ in cproj_gen(4 * (NQ - 2) + 3):
                pass
            flush_all()
            for tb in range(4 * (NQ - 1), 4 * NQ):
                for _ in cproj_gen(tb, split_dma=(tb == 4 * NQ - 1)):
                    pass

    nc.compile()
    return nc


def _get_nc():
    global _cached_nc
    if _cached_nc is None:
        _cached_nc = _build_nc()
    return _cached_nc


class _Runner:
    """Compile the bass module to a PJRT executable once, reuse across calls
    (run_bass_kernel_spmd re-jits a fresh closure every call, which costs
    seconds; this caches the jitted shard_map'd executable)."""

    def __init__(self, nc):
        import jax
        from jax.sharding import Mesh, PartitionSpec
        from jax.experimental.shard_map import shard_map
        import concourse.mybir as mybir
        from concourse.bass2jax import (
            _bass_exec_p, install_neuronx_cc_hook, partition_id_tensor,
        )

        install_neuronx_cc_hook()
        self.nc = nc
        partition_name = (
            nc.partition_id_tensor.name if nc.partition_id_tensor else None
        )
        in_names: list[str] = []
        out_names: list[str] = []
        out_avals = []
        zero_outs: list[np.ndarray] = []
        for alloc in nc.m.functions[0].allocations:
            if not isinstance(alloc, mybir.MemoryLocationSet):
                continue
            name = alloc.memorylocations[0].name
            if alloc.kind == "ExternalInput":
                if name != partition_name:
                    in_names.append(name)
            elif alloc.kind == "ExternalOutput":
                out_names.append(name)
                shape = tuple(alloc.tensor_shape)
                dtype = mybir.dt.np(alloc.dtype)
                out_avals.append(jax.core.ShapedArray(shape, dtype))
                zero_outs.append(np.zeros(shape, dtype))
        self.in_names = in_names
        self.out_names = out_names
        self.out_avals = out_avals
        n_params = len(in_names)
        n_outs = len(out_names)
        all_names = in_names + out_names
        if partition_name is not None:
            all_names = all_names + [partition_name]

        def _body(*args):
            operands = list(args)
            if partition_name is not None:
                operands.append(partition_id_tensor())
            outs = _bass_exec_p.bind(
                *operands,
                out_avals=tuple(out_avals),
                in_names=tuple(all_names),
                out_names=tuple(out_names),
                lowering_input_output_aliases=(),
                sim_require_finite=False,
                sim_require_nnan=False,
                nc=nc,
            )
            return tuple(outs)

        devices = jax.devices()[:N_CORES]
        assert len(devices) == N_CORES
        mesh = Mesh(np.asarray(devices), ("core",))
        self._sharding = jax.sharding.NamedSharding(mesh, PartitionSpec("core"))
        in_specs = (PartitionSpec("core"),) * (n_params + n_outs)
        out_specs = (PartitionSpec("core"),) * n_outs
        self._fn = jax.jit(
            shard_map(_body, mesh=mesh, in_specs=in_specs, out_specs=out_specs,
                      check_rep=False),
            keep_unused=True,
        )
        # The kernel writes every element of its outputs, so the "zero
        # output" operands are never read: stage them on device once instead
        # of shipping 64MB of zeros over the axon link per call.
        self._staged_zeros = [
            jax.device_put(
                np.zeros((N_CORES * z.shape[0], *z.shape[1:]), z.dtype),
                self._sharding)
            for z in zero_outs
        ]
        # Pairwise partial-sum on device: cores 2b and 2b+1 hold the two
        # half-head partials of batch b; adding them on-device halves the
        # bytes fetched over the slow axon link. Falls back to host if the
        # collective fails to compile/run.
        def _pairsum(o):
            import jax.numpy as jnp

            o = o.reshape(N_CORES, T, C).astype(jnp.float32)
            return o[0::2] + o[1::2]

        self._pairsum = jax.jit(_pairsum)
        self._use_dev_sum = True

    def __call__(self, in_maps):
        import jax

        concat_in = [
            np.concatenate([np.asarray(in_maps[c][n]) for c in range(N_CORES)],
                           axis=0)
            for n in self.in_names
        ]
        out_arrs = self._fn(*concat_in, *self._staged_zeros)
        out_g = out_arrs[0]
        if self._use_dev_sum:
            try:
                summed = np.asarray(self._pairsum(out_g))
                return {"summed": summed}
            except Exception:
                self._use_dev_sum = False
        full = np.asarray(out_g).reshape(N_CORES, T, C)
        return {"percore": full}


def _get_runner():
    global _runner
    if _runner is None:
        _runner = _Runner(_get_nc())
    return _runner


def _prep_inputs(x, W_attn, b_attn, W_proj):
    """Per-core input dicts; per-batch and per-group arrays computed once."""
    xts = [np.ascontiguousarray(x[b].T.astype(BF16)) for b in range(B)]
    eye = np.eye(128, dtype=BF16)
    per_g = []
    for g in range(2):
        gs = slice(g * GC, (g + 1) * GC)
        wqkv = np.ascontiguousarray(np.concatenate(
            [W_attn[:, 1 * C:2 * C][:, gs], W_attn[:, 0 * C:1 * C][:, gs],
             W_attn[:, 2 * C:3 * C][:, gs]], axis=1).astype(BF16))
        wp = np.ascontiguousarray(W_proj[g * GC:(g + 1) * GC, :].astype(BF16))
        bqk = np.ascontiguousarray(np.concatenate(
            [b_attn[1 * C:2 * C][gs].reshape(4, 128).T,
             b_attn[0 * C:1 * C][gs].reshape(4, 128).T,
             b_attn[2 * C:3 * C][gs].reshape(4, 128).T],
            axis=1).astype(np.float32))
        per_g.append({"wqkv": wqkv, "wp": wp, "bqk": bqk})
    return [
        {"xt": xts[c // 2], "ident": eye, **per_g[c % 2]}
        for c in range(N_CORES)
    ]


def kernel(x, W_attn, b_attn, W_proj, b_proj):
    global LAST_RESULTS
    x = np.asarray(x, dtype=np.float32)
    W_attn = np.asarray(W_attn, dtype=np.float32)
    b_attn = np.asarray(b_attn, dtype=np.float32)
    W_proj = np.asarray(W_proj, dtype=np.float32)
    b_proj = np.asarray(b_proj, dtype=np.float32)

    runner = _get_runner()
    in_maps = _prep_inputs(x, W_attn, b_attn, W_proj)
    res = runner(in_maps)
    LAST_RESULTS = res

    if "summed" in res:
        return res["summed"] + b_proj
    pc = res["percore"].astype(np.float32)
    full = np.empty((B, T, C), np.float32)
    for b in range(B):
        full[b] = pc[2 * b] + pc[2 * b + 1] + b_proj
    return full
